# revision 1
# baseline (speedup 1.0000x reference)
"""Trainium2 Bass kernel for nn_MultiHeadCrossAttention (B=4, T=1024, E=1024, H=16).

Sharding: the computation splits into 8 fully independent shards with zero
cross-core communication: (output stream s, batch b) for s in {1,2}, b in 0..3.
Stream-1 output xo@Wout1 needs K,V from x and Q from y; stream-2 the reverse.
Core c<4 computes stream-1 batch c; core c>=4 computes stream-2 batch c-4.

Per-core kernel (all activations kept transposed, feature-on-partition):
  Q^T = Wq^T.T @ B^T, K^T = Wk^T.T @ A^T   (fp32r matmuls, K=1024)
  V   = A^T.T @ Wv^T                        (natural layout, bf16 store)
  per head pair (2m, 2m+1), row-tiled K=64 matmuls:
    S^T[j,i] = K^T.T @ Q^T;  P^T = exp(S^T/8) on ACT (bf16)
    O'^T = V.T @ P^T (col-tiled M=64 pairs) ; rowsums via M=1 ones-matmuls
    recip = 1/rowsum (DVE); broadcast via K=1 ones-matmul; O^T = O'^T * bcast
  Z^T = Wout^T.T @ O^T  (fp32r, accumulate over head chunks)
Host side pre-transposes/groups weights and activations, and re-transposes
the per-core outputs.
"""

import os
import sys
import time

sys.path.insert(0, "/opt/trn_rl_repo")

import numpy as np
import ml_dtypes
from contextlib import ExitStack

import concourse.bass as bass
import concourse.mybir as mybir
import concourse.tile as tile
from concourse import bacc
from concourse import bass_utils

B, T, E, H = 4, 1024, 1024, 16
D = E // H            # 64
NC = E // 128         # 8 chunks of 128
NIC = T // 512        # 2 free-dim chunks of 512
N_CORES = 8

F32R = mybir.dt.float32r
F32 = mybir.dt.float32
BF16 = mybir.dt.bfloat16
F16 = mybir.dt.float16

_NC_CACHE = {}
LAST_RESULTS = {}

_SELBC = np.zeros((33, 128), np.float32)
_SELBC[0, 0:64] = 1.0
_SELBC[32, 64:128] = 1.0


KPHASE = os.environ.get("KPHASE", "PAZ")
KSKIP = set(os.environ.get("KSKIP", "").split(","))



def _build():
    KREP = int(os.environ.get("KREP", "1"))
    nc = bacc.Bacc("TRN2", target_bir_lowering=False, debug=False,
                   enable_asserts=False, num_devices=N_CORES)
    a_t = nc.dram_tensor("a_t", (E, T), F16, kind="ExternalInput").ap()
    b_t = nc.dram_tensor("b_t", (E, T), F16, kind="ExternalInput").ap()
    wq_t = nc.dram_tensor("wq_t", (E, E), F16, kind="ExternalInput").ap()
    wk_t = nc.dram_tensor("wk_t", (E, E), F16, kind="ExternalInput").ap()
    wv_t = nc.dram_tensor("wv_t", (E, E), F16, kind="ExternalInput").ap()
    wout_t = nc.dram_tensor("wout_t", (E, E), F16, kind="ExternalInput").ap()
    selbc_d = nc.dram_tensor("selbc", (33, 128), F16, kind="ExternalInput").ap()
    z_t = nc.dram_tensor("z_t", (E, T), F32, kind="ExternalOutput").ap()

    with tile.TileContext(nc) as tc, ExitStack() as ctx:
        # long-lived pools
        qkv_pool = ctx.enter_context(tc.tile_pool(name="qkv", bufs=1))
        const_pool = ctx.enter_context(tc.tile_pool(name="const", bufs=1))

        qt = qkv_pool.tile([128, NC, T], F16, tag="qt")
        kt = qkv_pool.tile([128, NC, T], F16, tag="kt")
        v = qkv_pool.tile([128, NC, H * (D + 1)], F16, tag="v")

        for _m in range(NC):
            nc.vector.memset(
                v[:, _m, :].rearrange("p (h x) -> p h x", x=D + 1)[:, :, D:D + 1], 1.0)
        if KSKIP & {"proj"}:
            for m in range(NC):
                nc.vector.memset(qt[:, m, :], 0.25)
                nc.vector.memset(kt[:, m, :], 0.25)
                nc.vector.memset(v[:, m, :], 0.25)
        selbc = const_pool.tile([33, 128], F16, tag="selbc")
        nc.sync.dma_start(selbc[:], selbc_d)

        # ---------------- Phase P: projections ----------------
        for _rep in range(KREP):
            _build_body(nc, tc, ctx, locals())
    nc.compile()
    return nc


def _build_body(nc, tc, ctx, env):
    qt, kt, v = env["qt"], env["kt"], env["v"]
    selbc = env["selbc"]
    a_t, b_t = env["a_t"], env["b_t"]
    wq_t, wk_t, wv_t, wout_t = env["wq_t"], env["wk_t"], env["wv_t"], env["wout_t"]
    z_t = env["z_t"]
    if True:
        with tc.tile_pool(name="acts", bufs=1) as acts, \
             tc.tile_pool(name="pps", bufs=3, space="PSUM") as pps:
            at_sb = acts.tile([128, NC, T], F16, tag="at")
            bt_sb = acts.tile([128, NC, T], F16, tag="bt")
            wv_sb = acts.tile([128, NC, E], F16, tag="wv")
            wq_sb = acts.tile([128, NC, E], F16, tag="wq")
            wk_sb = acts.tile([128, NC, E], F16, tag="wk")
            # issue order matters: Q^T-proj (bt, wq) starts first
            for c in range(NC):
                nc.sync.dma_start(bt_sb[:, c, :], b_t[c * 128:(c + 1) * 128, :])
                nc.sync.dma_start(wq_sb[:, c, :], wq_t[c * 128:(c + 1) * 128, :])
            for c in range(NC):
                nc.sync.dma_start(at_sb[:, c, :], a_t[c * 128:(c + 1) * 128, :])
                nc.sync.dma_start(wk_sb[:, c, :], wk_t[c * 128:(c + 1) * 128, :])
            for c in range(NC):
                nc.sync.dma_start(wv_sb[:, c, :], wv_t[c * 128:(c + 1) * 128, :])

            # Q^T and K^T: out[dh-chunk m][t] = sum_e w[e, dh] * act[e, t]
            # weight column-blocks streamed per m (each block used exactly once)
            for (w_sb, act_sb, out_sb) in (
                () if "proj" in KSKIP else (
                (wq_sb, bt_sb, qt),
                (wk_sb, at_sb, kt),
            )):
                for m in range(NC):
                    ps = pps.tile([128, T], F32, tag="pp")
                    for e in range(NC):
                        for ic in range(NIC):
                            nc.tensor.matmul(
                                ps[:, bass.ts(ic, 512)],
                                w_sb[:, e, bass.ts(m, 128)],
                                act_sb[:, e, bass.ts(ic, 512)],
                                start=(e == 0), stop=(e == NC - 1))
                    nc.scalar.copy(out_sb[:, m, :], ps[:])
            # V natural: out[j-chunk][dv] = sum_e at[e, j] * wv[e, dv]
            for m in range(NC) if "proj" not in KSKIP else ():
                ps = pps.tile([128, T], F32, tag="pp")
                for e in range(NC):
                    for ic in range(NIC):
                        nc.tensor.matmul(
                            ps[:, bass.ts(ic, 512)],
                            at_sb[:, e, bass.ts(m, 128)],
                            wv_sb[:, e, bass.ts(ic, 512)],
                            start=(e == 0), stop=(e == NC - 1))
                nc.scalar.copy(
                    v[:, m, :].rearrange("p (h x) -> p h x", x=D + 1)[:, :, 0:D],
                    ps[:].rearrange("p (h x) -> p h x", x=D))

        if KPHASE == "P":
            with tc.tile_pool(name="zdbg", bufs=2) as zdbgp:
                for cc in range(NC):
                    zdbg = zdbgp.tile([128, T], F32, tag="zdbg")
                    nc.vector.tensor_copy(zdbg[:], qt[:, cc, :])
                    nc.sync.dma_start(z_t[cc * 128:(cc + 1) * 128, :], zdbg[:])
            return

        # ---------------- Phase A: attention per head pair ----------------
        rep_ctx = ctx.enter_context(ExitStack())
        shps = rep_ctx.enter_context(tc.tile_pool(name="shps", bufs=2, space="PSUM"))
        ot_pool = rep_ctx.enter_context(tc.tile_pool(name="ot", bufs=1))
        ot = ot_pool.tile([128, NC, T], F16, tag="ot")
        with tc.tile_pool(name="pt", bufs=4) as ptp, \
             tc.tile_pool(name="nrm", bufs=4) as nrm, \
             tc.tile_pool(name="nrm8", bufs=8) as nrm8, \
             tc.tile_pool(name="ops", bufs=2, space="PSUM") as ops:
            for m in range(NC):
                ptA = ptp.tile([128, NC, T], F16, tag="pt")
                ptB = ptp.tile([128, NC, T], F16, tag="pt")
                if "sexp" in KSKIP:
                    nc.vector.memset(ptA[:], 1.0)
                    nc.vector.memset(ptB[:], 1.0)
                for jc in range(NC) if "sexp" not in KSKIP else ():
                    ps_s = shps.tile([128, T], F32, tag="big")
                    ps_sB = shps.tile([128, T], F32, tag="big")
                    for ic in range(NIC):
                        nc.tensor.matmul(
                            ps_s[:, bass.ts(ic, 512)],
                            kt[0:64, m, bass.ts(jc, 128)],
                            qt[0:64, m, bass.ts(ic, 512)],
                            start=True, stop=True)
                        nc.tensor.matmul(
                            ps_sB[:, bass.ts(ic, 512)],
                            kt[64:128, m, bass.ts(jc, 128)],
                            qt[64:128, m, bass.ts(ic, 512)],
                            start=True, stop=True, tile_position=(64, 0))
                    nc.scalar.activation(ptA[:, jc, :], ps_s[:],
                                         mybir.ActivationFunctionType.Exp, scale=0.125)
                    nc.scalar.activation(ptB[:, jc, :], ps_sB[:],
                                         mybir.ActivationFunctionType.Exp, scale=0.125)

                if "ovr" in KSKIP:
                    nc.vector.memset(ot[:, m, :], 0.25)
                    continue
                ps_oA = ops.tile([65, T], F32, tag="o")
                ps_oB = ops.tile([65, T], F32, tag="o")
                hA, hB = 2 * m, 2 * m + 1
                for jc in range(NC):
                    st = dict(start=(jc == 0), stop=(jc == NC - 1))
                    for ic in range(NIC):
                        s_ic = bass.ts(ic, 512)
                        nc.tensor.matmul(ps_oA[:, s_ic], v[:, jc, bass.ts(hA, D + 1)],
                                         ptA[:, jc, s_ic], **st)
                        nc.tensor.matmul(ps_oB[:, s_ic], v[:, jc, bass.ts(hB, D + 1)],
                                         ptB[:, jc, s_ic], **st)

                recip2 = nrm.tile([33, T], F16, tag="recip2")
                nc.vector.memset(recip2[:], 0.0)
                with nc.allow_low_precision(reason="recip feeds fp16 bc matmul"):
                    nc.vector.reciprocal(recip2[0:1, :], ps_oA[64:65, :])
                    nc.vector.reciprocal(recip2[32:33, :], ps_oB[64:65, :])
                ps_bcA = shps.tile([64, T], F32, tag="big")
                ps_bcB = shps.tile([64, T], F32, tag="big")
                for ic in range(NIC):
                    s_ic = bass.ts(ic, 512)
                    nc.tensor.matmul(ps_bcA[:, s_ic], selbc[:, 0:64], recip2[:, s_ic],
                                     start=True, stop=True)
                    nc.tensor.matmul(ps_bcB[:, s_ic], selbc[:, 64:128], recip2[:, s_ic],
                                     start=True, stop=True)
                bcA = nrm.tile([64, T], F32, tag="bcA")
                bcB = nrm.tile([64, T], F32, tag="bcB")
                nc.scalar.copy(bcA[:], ps_bcA[:])
                nc.scalar.copy(bcB[:], ps_bcB[:])
                with nc.allow_low_precision(reason="O^T fp16 feeds fp16 out-proj"):
                    nc.vector.tensor_mul(ot[0:64, m, :], ps_oA[0:64, :], bcA[:])
                    nc.vector.tensor_mul(ot[64:128, m, :], ps_oB[0:64, :], bcB[:])

        if KPHASE == "PA":
            with tc.tile_pool(name="zdbg", bufs=2) as zdbgp:
                for cc in range(NC):
                    zdbg = zdbgp.tile([128, T], F32, tag="zdbg")
                    nc.vector.tensor_copy(zdbg[:], ot[:, cc, :])
                    nc.sync.dma_start(z_t[cc * 128:(cc + 1) * 128, :], zdbg[:])
            return

        # ---------------- Phase Z: out-projection ----------------
        with tc.tile_pool(name="wout", bufs=1) as woutp, \
             tc.tile_pool(name="zsb", bufs=2) as zsbp, \
             tc.tile_pool(name="zps", bufs=2, space="PSUM") as zps:
            wo = woutp.tile([128, NC, E], F16, tag="wo")
            for c in range(NC):
                nc.sync.dma_start(wo[:, c, :], wout_t[c * 128:(c + 1) * 128, :])
            for cc in range(NC):
                ps = zps.tile([128, T], F32, tag="z")
                for m in range(NC):
                    for ic in range(NIC):
                        nc.tensor.matmul(
                            ps[:, bass.ts(ic, 512)],
                            wo[:, m, bass.ts(cc, 128)],
                            ot[:, m, bass.ts(ic, 512)],
                            start=(m == 0), stop=(m == NC - 1))
                zsb = zsbp.tile([128, T], F32, tag="zsb")
                nc.scalar.copy(zsb[:], ps[:])
                nc.sync.dma_start(z_t[cc * 128:(cc + 1) * 128, :], zsb[:])
        rep_ctx.close()


def _group_w(wqkv, k):
    """Rows of Wqkv (3E, E) for q/k/v (k=0/1/2), grouped head-major.

    Row index layout: r = di*(3H) + k*H + h  ->  grouped[h*D+di, :].
    """
    w = np.asarray(wqkv, dtype=np.float32).reshape(D, 3, H, E)[:, k]   # [di, h, e]
    return np.ascontiguousarray(w.transpose(1, 0, 2).reshape(E, E))    # [h*D+di, e]


def kernel(x, y, Wqkv1, Wqkv2, Wout1, Wout2):
    x = np.asarray(x, dtype=np.float32)
    y = np.asarray(y, dtype=np.float32)

    if "nc" not in _NC_CACHE:
        _NC_CACHE["nc"] = _build()
    nc = _NC_CACHE["nc"]

    # weight prep (host): grouped + transposed (fp16 on-device dtype)
    wq1_t = np.ascontiguousarray(_group_w(Wqkv1, 0).T)
    wk1_t = np.ascontiguousarray(_group_w(Wqkv1, 1).T)
    wv1_t = np.ascontiguousarray(_group_w(Wqkv1, 2).T)
    wq2_t = np.ascontiguousarray(_group_w(Wqkv2, 0).T)
    wk2_t = np.ascontiguousarray(_group_w(Wqkv2, 1).T)
    wv2_t = np.ascontiguousarray(_group_w(Wqkv2, 2).T)
    wout1_t = np.ascontiguousarray(np.asarray(Wout1, dtype=np.float32).T)
    wout2_t = np.ascontiguousarray(np.asarray(Wout2, dtype=np.float32).T)

    in_maps = []
    for c in range(N_CORES):
        s, b = divmod(c, B)
        if s == 0:
            # stream-1 output: K,V from x via Wqkv1; Q from y via Wqkv2
            a_t, b_t = x[b].T, y[b].T
            wq, wk, wv, wo = wq2_t, wk1_t, wv1_t, wout1_t
        else:
            a_t, b_t = y[b].T, x[b].T
            wq, wk, wv, wo = wq1_t, wk2_t, wv2_t, wout2_t
        in_maps.append({
            "a_t": np.ascontiguousarray(a_t).astype(np.float16),
            "b_t": np.ascontiguousarray(b_t).astype(np.float16),
            "wq_t": wq.astype(np.float16), "wk_t": wk.astype(np.float16),
            "wv_t": wv.astype(np.float16), "wout_t": wo.astype(np.float16),
            "selbc": _SELBC.astype(np.float16),
        })

    trace = os.environ.get("BASS_KERNEL_TRACE", "0") == "1"
    if trace:
        try:
            from antenv.axon_hooks import get_axon_ntff_profile_hook  # noqa: F401
        except ImportError:
            trace = False
    ncores = int(os.environ.get("KCORES", str(N_CORES)))
    r = bass_utils.run_bass_kernel_spmd(nc, in_maps[:ncores], core_ids=list(range(ncores)),
                                        trace=trace)
    LAST_RESULTS["exec_time_ns"] = r.exec_time_ns
    LAST_RESULTS["profile_json"] = r.profile_json

    out1 = np.stack([r.results[b]["z_t"].T for b in range(B)]).astype(np.float32)
    out2 = np.stack([r.results[B + b]["z_t"].T for b in range(B)]).astype(np.float32)
    return out1, out2

